# revision 25
# baseline (speedup 1.0000x reference)
"""DETR-style stroke loss on 8 TRN2 NeuronCores.

Strategy (pure data parallel, batch sharded 8 ways, 32 samples/core):
 - Host: Hungarian matching per sample (sequential + data-dependent, so it
   stays on host), then gather matched pred/gt rows and fold the per-sample
   normalization weights (coord/width scales, match mask, 1/B mean) into two
   dense tensors A, B such that sum |A - B| == the full weighted L1 term.
 - Device (raw Bass, explicit semaphores): two parallel HWDGE DMAs bring
   A|pred_validity and B|gt_validity ([128,176] f32 each); DVE computes the
   L1 partials; ACT computes the BCE logs (Ln table pre-warmed during the
   DMAs, torch-style -100 clamp on log(pv) only — log(1-pv) cannot reach
   -100 for f32 pv in [0,1)); one PE matmul against a memset ones vector
   reduces all per-partition partials to a [1,3] scalar triple.
 - Host: combine the 8 per-core triples into the scalar loss (the
   "all-reduce-mean" step).

Per-core device traffic = the exact input footprint (22 f32 per (b,s)),
i.e. the memory roofline for this loss. HW exec ~15.4 us, dominated by
fixed NEFF preamble/barrier/DMA-latency costs (~13 us).
"""

import numpy as np

B, S, D = 256, 64, 10
NCORES = 8
BL = B // NCORES            # 32 samples per core
ROWS = 128                  # SBUF partitions
LW = BL * S * D // ROWS     # 160 f32 of A (and of B) per partition
VW = BL * S // ROWS         # 16 f32 of pv (and of gv) per partition
XW = 2 * LW + 2 * VW + 1    # 353: A | B | pv | gv | ones

_CACHE = {}
LAST_RESULTS = None         # BassKernelResults of the most recent run


# ---------------------------------------------------------------- matching

def _hungarian_fallback(cost):
    """Jonker-Volgenant (e-maxx) linear sum assignment, scipy semantics.
    cost: [R, C] float64. Returns (row_ind, col_ind), min(R, C) pairs."""
    cost = np.asarray(cost, dtype=np.float64)
    transposed = cost.shape[0] > cost.shape[1]
    Cm = cost.T if transposed else cost
    n, m = Cm.shape  # n <= m
    u = np.zeros(n + 1)
    v = np.zeros(m + 1)
    p = np.zeros(m + 1, dtype=np.int64)
    way = np.zeros(m + 1, dtype=np.int64)
    for i in range(1, n + 1):
        p[0] = i
        j0 = 0
        minv = np.full(m + 1, np.inf)
        used = np.zeros(m + 1, dtype=bool)
        while True:
            used[j0] = True
            i0 = p[j0]
            jf = np.nonzero(~used[1:])[0] + 1
            cur = Cm[i0 - 1, jf - 1] - u[i0] - v[jf]
            upd = cur < minv[jf]
            minv[jf] = np.where(upd, cur, minv[jf])
            way[jf[upd]] = j0
            j1 = jf[np.argmin(minv[jf])]
            delta = minv[j1]
            u[p[used]] += delta
            v[used] -= delta
            minv[jf] -= delta
            j0 = j1
            if p[j0] == 0:
                break
        while j0:
            j1 = way[j0]
            p[j0] = p[j1]
            j0 = j1
    col4row = np.zeros(n, dtype=np.int64)
    for j in range(1, m + 1):
        if p[j] > 0:
            col4row[p[j] - 1] = j - 1
    rows = np.arange(n, dtype=np.int64)
    return (col4row, rows) if transposed else (rows, col4row)


def _lsap(cost):
    try:
        from scipy.optimize import linear_sum_assignment
        return linear_sum_assignment(cost)
    except ImportError:
        return _hungarian_fallback(cost)


def _compute_matching(ps, tg):
    """Per-sample optimal assignment. Returns P, G [B,S] int64, M [B,S] f32,
    N [B] f32 (matches the reference layout)."""
    P = np.zeros((B, S), dtype=np.int64)
    G = np.zeros((B, S), dtype=np.int64)
    M = np.zeros((B, S), dtype=np.float32)
    N = np.zeros((B,), dtype=np.float32)
    for b in range(B):
        valid = np.nonzero(tg[b, :, 10] > 0.5)[0]
        ng = valid.size
        N[b] = ng
        if ng == 0:
            continue
        gt = tg[b, valid, :10]
        d = np.abs(ps[b][:, None, :] - gt[None])          # [S, ng, 10] f32
        cost = 5.0 * d[..., :8].sum(-1) + d[..., 8:10].sum(-1)
        ri, ci = _lsap(cost)
        P[b, :ng] = ri
        G[b, :ng] = valid[ci]
        M[b, :ng] = 1.0
    return P, G, M, N


# ---------------------------------------------------------------- device

def _build_nc():
    """Raw Bass (no Tile): explicit semaphores, overlap-oriented.

    Two balanced input DMAs on the two HWDGE rings run in parallel:
    sync (SP) brings A|pv, scalar (ACT) brings B|gv — no small-descriptor
    tail DMA. The ones vector for the PE reduction and the Ln-table warmup
    input come from one DVE memset tile. The Ln PWP table is pre-warmed
    while the DMAs fly. sum(log1m) comes from a third Ln's accum_out so its
    ACTIVATION_READ_ACCUMULATOR stays off the BCE critical path. All three
    per-partition partial columns land in one [128,3] tile that a single PE
    matmul against the ones vector reduces to [1,3] in PSUM -> SBUF -> one
    12-byte output DMA.
    """
    import concourse.bass as bass
    import concourse.mybir as mybir

    f32 = mybir.dt.float32
    X = mybir.AxisListType.X
    ADD = mybir.AluOpType.add
    LN = mybir.ActivationFunctionType.Ln

    W1 = LW + VW             # 176 cols: A | pv
    W2 = LW + VW             # 176 cols: B | gv

    nc = bass.Bass(enable_partition_id=False)
    x1 = nc.declare_dram_parameter("x1", [ROWS, W1], f32, isOutput=False)
    x2 = nc.declare_dram_parameter("x2", [ROWS, W2], f32, isOutput=False)
    out = nc.declare_dram_parameter("out", [1, 3], f32, isOutput=True)

    with (
        nc.sbuf_tensor([ROWS, W1], f32) as xa,
        nc.sbuf_tensor([ROWS, W2], f32) as xb,
        nc.sbuf_tensor([ROWS, LW], f32) as d,
        nc.sbuf_tensor([ROWS, VW], f32) as logp,
        nc.sbuf_tensor([ROWS, VW], f32) as log1m,
        nc.sbuf_tensor([ROWS, VW], f32) as log1mb,
        nc.sbuf_tensor([ROWS, VW], f32) as dlog,
        nc.sbuf_tensor([ROWS, VW], f32) as sbce,
        nc.sbuf_tensor([ROWS, 3], f32) as parts,
        nc.sbuf_tensor([ROWS, 1], f32) as ones,
        nc.sbuf_tensor([ROWS, 1], f32) as warmout,
        nc.sbuf_tensor([1, 3], f32) as res,
        nc.psum_tensor([1, 3], f32) as acc,
        nc.semaphore("s_a") as s_a,
        nc.semaphore("s_b") as s_b,
        nc.semaphore("s_g") as s_g,
        nc.semaphore("s_act") as s_act,
        nc.semaphore("s_v") as s_v,
        nc.semaphore("s_pe") as s_pe,
        nc.Block() as block,
    ):
        a_ap = xa[:, 0:LW]
        pv_ap = xa[:, LW:W1]
        b_ap = xb[:, 0:LW]
        gv_ap = xb[:, LW:W2]

        i32 = mybir.dt.int32

        @block.sync
        def _(sync):
            sync.dma_start(out=xa[:], in_=x1[:]).then_inc(s_a, 16)
            sync.wait_ge(s_v, 7)
            # 12-byte result via sequencer register stores: a DMA round trip
            # for this is ~2.6us (issue + completion sem); three posted WRITEs
            # retire in ~0.1us and are fenced by the end-of-stream drain.
            with (
                sync.register("r0") as r0,
                sync.register("r1") as r1,
                sync.register("r2") as r2,
            ):
                sync.reg_load([r0, r1, r2], res[0:1, 0:3].bitcast(i32))
                out_i = out.bitcast(i32)
                sync.reg_save(out_i[0:1, 0:1], r0)
                sync.reg_save(out_i[0:1, 1:2], r1)
                sync.reg_save(out_i[0:1, 2:3], r2)

        @block.scalar
        def _(scalar):
            scalar.dma_start(out=xb[:], in_=x2[:]).then_inc(s_b, 16)
            # warm the Ln PWP table while the input DMAs are in flight
            scalar.wait_ge(s_g, 1)
            scalar.activation(warmout[:], ones[:], LN)
            scalar.wait_ge(s_a, 16)
            scalar.activation(logp[:], pv_ap, LN).then_inc(s_act, 1)
            scalar.activation(log1m[:], pv_ap, LN, bias=1.0, scale=-1.0).then_inc(
                s_act, 1
            )
            # third Ln only exists to produce sum(log1m) via the accumulator,
            # keeping READ_ACCUMULATOR off the logp/log1m critical path
            scalar.activation(
                log1mb[:], pv_ap, LN, bias=1.0, scale=-1.0,
                accum_out=parts[:, 2:3],
            ).then_inc(s_act, 1)

        @block.vector
        def _(vector):
            vector.memset(ones[:], 1.0).then_inc(s_g, 1)
            # weighted L1: per-partition sum of |A - B|
            vector.wait_ge(s_a, 16)
            vector.wait_ge(s_b, 16)
            vector.tensor_sub(d[:], a_ap, b_ap).then_inc(s_v, 1)             # ->1
            # clamp interleaved here: it only needs Ln1, which lands mid-L1
            vector.wait_ge(s_act, 1)
            vector.tensor_scalar_max(logp[:], logp[:], -100.0).then_inc(s_v, 1)  # ->2
            vector.wait_ge(s_v, 1)
            vector.tensor_reduce(
                parts[:, 0:1], d[:], axis=X, op=ADD, apply_absolute_value=True
            ).then_inc(s_v, 1)                                               # ->3
            # BCE: sum -(gv*logp + (1-gv)*log1m)
            #    = -(sum gv*(logp-log1m) + sum log1m); third term via ACT accum
            vector.wait_ge(s_act, 2)
            vector.wait_ge(s_v, 2)
            vector.tensor_sub(dlog[:], logp[:], log1m[:]).then_inc(s_v, 1)   # ->4
            vector.wait_ge(s_v, 4)
            vector.tensor_mul(sbce[:], gv_ap, dlog[:]).then_inc(s_v, 1)      # ->5
            vector.wait_ge(s_v, 5)
            vector.tensor_reduce(
                parts[:, 1:2], sbce[:], axis=X, op=ADD
            ).then_inc(s_v, 1)                                               # ->6
            # PSUM [1,3] -> SBUF for the output DMA
            vector.wait_ge(s_pe, 1)
            vector.tensor_copy(res[:], acc[:]).then_inc(s_v, 1)              # ->7

        @block.tensor
        def _(tensor):
            # cross-partition reduction of all three partial columns at once
            tensor.wait_ge(s_g, 1)
            tensor.wait_ge(s_act, 3)
            tensor.wait_ge(s_v, 6)
            tensor.matmul(
                acc[:], ones[:], parts[:], start=True, stop=True
            ).then_inc(s_pe, 1)

    return nc


def _get_nc():
    if "nc" not in _CACHE:
        _CACHE["nc"] = _build_nc()
    return _CACHE["nc"]


# ---------------------------------------------------------------- host prep

def _build_in_maps(ps, pv, tg):
    P, G, M, N = _compute_matching(ps, tg)
    mp = np.take_along_axis(ps, P[..., None], axis=1)          # [B,S,10]
    mg = np.take_along_axis(tg[..., :10], G[..., None], axis=1)
    ng = np.maximum(N, 1.0)                                    # [B]
    w = np.empty((B, 1, D), np.float32)
    w[:, 0, :8] = (5.0 / (8.0 * ng * B))[:, None]
    w[:, 0, 8:] = (1.0 / (2.0 * ng * B))[:, None]
    wm = w * M[..., None]
    A = (mp * wm).astype(np.float32)
    Bt = (mg * wm).astype(np.float32)
    gv = np.ascontiguousarray(tg[..., 10:11])

    in_maps = []
    for i in range(NCORES):
        sl = slice(i * BL, (i + 1) * BL)
        x1 = np.concatenate(
            [A[sl].reshape(ROWS, LW), pv[sl].reshape(ROWS, VW)], axis=1
        )
        x2 = np.concatenate(
            [Bt[sl].reshape(ROWS, LW), gv[sl].reshape(ROWS, VW)], axis=1
        )
        in_maps.append(
            {"x1": np.ascontiguousarray(x1), "x2": np.ascontiguousarray(x2)}
        )
    return in_maps


def _combine(outs):
    """outs: per-core [1,3] = [sum weighted-L1, sum gv*dlog, sum log1m]."""
    total = np.float64(0.0)
    for r in outs:
        r = np.asarray(r, dtype=np.float64)
        total += r[0, 0] - (r[0, 1] + r[0, 2]) / float(S * B)
    return total


def kernel(pred_strokes, pred_validity, targets):
    global LAST_RESULTS
    from concourse.bass_utils import run_bass_kernel_spmd

    ps = np.asarray(pred_strokes, dtype=np.float32)
    pv = np.asarray(pred_validity, dtype=np.float32)
    tg = np.asarray(targets, dtype=np.float32)

    in_maps = _build_in_maps(ps, pv, tg)
    nc = _get_nc()
    LAST_RESULTS = run_bass_kernel_spmd(nc, in_maps, list(range(NCORES)))
    total = _combine([LAST_RESULTS.results[i]["out"] for i in range(NCORES)])
    return np.asarray(total, dtype=np.float32)


# revision 27
# speedup vs baseline: 1.1563x; 1.1563x over previous
"""DETR-style stroke loss on 8 TRN2 NeuronCores.

Strategy (pure data parallel, batch sharded 8 ways, 32 samples/core):
 - Host: Hungarian matching per sample (sequential + data-dependent, so it
   stays on host), then gather matched pred/gt rows and fold the per-sample
   normalization weights (coord/width scales, match mask, 1/B mean) into two
   dense tensors A, B such that sum |A - B| == the full weighted L1 term.
 - Device (raw Bass, explicit semaphores): two parallel HWDGE DMAs bring
   A|pred_validity and B|gt_validity ([128,176] f32 each); DVE computes the
   L1 partials; ACT computes the BCE logs (Ln table pre-warmed during the
   DMAs, torch-style -100 clamp on log(pv) only — log(1-pv) cannot reach
   -100 for f32 pv in [0,1)); one PE matmul against a memset ones vector
   reduces all per-partition partials to a [1,3] scalar triple.
 - Host: combine the 8 per-core triples into the scalar loss (the
   "all-reduce-mean" step).

Per-core device traffic = the exact input footprint (22 f32 per (b,s)),
i.e. the memory roofline for this loss. HW exec ~15.4 us, dominated by
fixed NEFF preamble/barrier/DMA-latency costs (~13 us).
"""

import numpy as np

B, S, D = 256, 64, 10
NCORES = 8
BL = B // NCORES            # 32 samples per core
ROWS = 128                  # SBUF partitions
LW = BL * S * D // ROWS     # 160 f32 of A (and of B) per partition
VW = BL * S // ROWS         # 16 f32 of pv (and of gv) per partition
XW = 2 * LW + 2 * VW + 1    # 353: A | B | pv | gv | ones

_CACHE = {}
LAST_RESULTS = None         # BassKernelResults of the most recent run


# ---------------------------------------------------------------- matching

def _hungarian_fallback(cost):
    """Jonker-Volgenant (e-maxx) linear sum assignment, scipy semantics.
    cost: [R, C] float64. Returns (row_ind, col_ind), min(R, C) pairs."""
    cost = np.asarray(cost, dtype=np.float64)
    transposed = cost.shape[0] > cost.shape[1]
    Cm = cost.T if transposed else cost
    n, m = Cm.shape  # n <= m
    u = np.zeros(n + 1)
    v = np.zeros(m + 1)
    p = np.zeros(m + 1, dtype=np.int64)
    way = np.zeros(m + 1, dtype=np.int64)
    for i in range(1, n + 1):
        p[0] = i
        j0 = 0
        minv = np.full(m + 1, np.inf)
        used = np.zeros(m + 1, dtype=bool)
        while True:
            used[j0] = True
            i0 = p[j0]
            jf = np.nonzero(~used[1:])[0] + 1
            cur = Cm[i0 - 1, jf - 1] - u[i0] - v[jf]
            upd = cur < minv[jf]
            minv[jf] = np.where(upd, cur, minv[jf])
            way[jf[upd]] = j0
            j1 = jf[np.argmin(minv[jf])]
            delta = minv[j1]
            u[p[used]] += delta
            v[used] -= delta
            minv[jf] -= delta
            j0 = j1
            if p[j0] == 0:
                break
        while j0:
            j1 = way[j0]
            p[j0] = p[j1]
            j0 = j1
    col4row = np.zeros(n, dtype=np.int64)
    for j in range(1, m + 1):
        if p[j] > 0:
            col4row[p[j] - 1] = j - 1
    rows = np.arange(n, dtype=np.int64)
    return (col4row, rows) if transposed else (rows, col4row)


def _lsap(cost):
    try:
        from scipy.optimize import linear_sum_assignment
        return linear_sum_assignment(cost)
    except ImportError:
        return _hungarian_fallback(cost)


def _compute_matching(ps, tg):
    """Per-sample optimal assignment. Returns P, G [B,S] int64, M [B,S] f32,
    N [B] f32 (matches the reference layout)."""
    P = np.zeros((B, S), dtype=np.int64)
    G = np.zeros((B, S), dtype=np.int64)
    M = np.zeros((B, S), dtype=np.float32)
    N = np.zeros((B,), dtype=np.float32)
    for b in range(B):
        valid = np.nonzero(tg[b, :, 10] > 0.5)[0]
        ng = valid.size
        N[b] = ng
        if ng == 0:
            continue
        gt = tg[b, valid, :10]
        d = np.abs(ps[b][:, None, :] - gt[None])          # [S, ng, 10] f32
        cost = 5.0 * d[..., :8].sum(-1) + d[..., 8:10].sum(-1)
        ri, ci = _lsap(cost)
        P[b, :ng] = ri
        G[b, :ng] = valid[ci]
        M[b, :ng] = 1.0
    return P, G, M, N


# ---------------------------------------------------------------- device

def _build_nc():
    """Raw Bass (no Tile): explicit semaphores, overlap-oriented.

    Two balanced input DMAs on the two HWDGE rings run in parallel:
    sync (SP) brings A|pv, scalar (ACT) brings B|gv — no small-descriptor
    tail DMA. The ones vector for the PE reduction and the Ln-table warmup
    input come from one DVE memset tile. The Ln PWP table is pre-warmed
    while the DMAs fly. sum(log1m) comes from a third Ln's accum_out so its
    ACTIVATION_READ_ACCUMULATOR stays off the BCE critical path. All three
    per-partition partial columns land in one [128,3] tile that a single PE
    matmul against the ones vector reduces to [1,3] in PSUM -> SBUF -> one
    12-byte output DMA.
    """
    import concourse.bass as bass
    import concourse.mybir as mybir

    f32 = mybir.dt.float32
    X = mybir.AxisListType.X
    ADD = mybir.AluOpType.add
    LN = mybir.ActivationFunctionType.Ln

    W1 = LW + VW             # 176 cols: A | pv
    W2 = LW + VW             # 176 cols: B | gv

    nc = bass.Bass(enable_partition_id=False)
    x1 = nc.declare_dram_parameter("x1", [ROWS, W1], f32, isOutput=False)
    x2 = nc.declare_dram_parameter("x2", [ROWS, W2], f32, isOutput=False)
    out = nc.declare_dram_parameter("out", [1, 3], f32, isOutput=True)

    with (
        nc.sbuf_tensor([ROWS, W1], f32) as xa,
        nc.sbuf_tensor([ROWS, W2], f32) as xb,
        nc.sbuf_tensor([ROWS, LW], f32) as d,
        nc.sbuf_tensor([ROWS, VW], f32) as logp,
        nc.sbuf_tensor([ROWS, VW], f32) as log1m,
        nc.sbuf_tensor([ROWS, VW], f32) as log1mb,
        nc.sbuf_tensor([ROWS, VW], f32) as dlog,
        nc.sbuf_tensor([ROWS, VW], f32) as sbce,
        nc.sbuf_tensor([ROWS, 3], f32) as parts,
        nc.sbuf_tensor([ROWS, 1], f32) as ones,
        nc.sbuf_tensor([ROWS, 1], f32) as warmout,
        nc.sbuf_tensor([1, 3], f32) as res,
        nc.psum_tensor([1, 3], f32) as acc,
        nc.semaphore("s_a") as s_a,
        nc.semaphore("s_b") as s_b,
        nc.semaphore("s_g") as s_g,
        nc.semaphore("s_act") as s_act,
        nc.semaphore("s_v") as s_v,
        nc.semaphore("s_pe") as s_pe,
        nc.semaphore("s_out") as s_out,
        nc.Block() as block,
    ):
        a_ap = xa[:, 0:LW]
        pv_ap = xa[:, LW:W1]
        b_ap = xb[:, 0:LW]
        gv_ap = xb[:, LW:W2]

        @block.sync
        def _(sync):
            sync.dma_start(out=xa[:], in_=x1[:]).then_inc(s_a, 16)
            sync.wait_ge(s_v, 7)
            # Register stores were tried here and are SLOWER: each reg_save to
            # a DRAM parameter loads the tensor's base address from the NEFF
            # relocation table (~0.9us TENSOR_LOAD, serialized per store).
            sync.dma_start(out=out[:], in_=res[:]).then_inc(s_out, 16)
            sync.wait_ge(s_out, 16)

        @block.scalar
        def _(scalar):
            scalar.dma_start(out=xb[:], in_=x2[:]).then_inc(s_b, 16)
            # warm the Ln PWP table while the input DMAs are in flight
            scalar.wait_ge(s_g, 1)
            scalar.activation(warmout[:], ones[:], LN)
            scalar.wait_ge(s_a, 16)
            scalar.activation(logp[:], pv_ap, LN).then_inc(s_act, 1)
            scalar.activation(log1m[:], pv_ap, LN, bias=1.0, scale=-1.0).then_inc(
                s_act, 1
            )
            # third Ln only exists to produce sum(log1m) via the accumulator,
            # keeping READ_ACCUMULATOR off the logp/log1m critical path
            scalar.activation(
                log1mb[:], pv_ap, LN, bias=1.0, scale=-1.0,
                accum_out=parts[:, 2:3],
            ).then_inc(s_act, 1)

        @block.vector
        def _(vector):
            vector.memset(ones[:], 1.0).then_inc(s_g, 1)
            # weighted L1: per-partition sum of |A - B|
            vector.wait_ge(s_a, 16)
            vector.wait_ge(s_b, 16)
            vector.tensor_sub(d[:], a_ap, b_ap).then_inc(s_v, 1)             # ->1
            # clamp interleaved here: it only needs Ln1, which lands mid-L1
            vector.wait_ge(s_act, 1)
            vector.tensor_scalar_max(logp[:], logp[:], -100.0).then_inc(s_v, 1)  # ->2
            vector.wait_ge(s_v, 1)
            vector.tensor_reduce(
                parts[:, 0:1], d[:], axis=X, op=ADD, apply_absolute_value=True
            ).then_inc(s_v, 1)                                               # ->3
            # BCE: sum -(gv*logp + (1-gv)*log1m)
            #    = -(sum gv*(logp-log1m) + sum log1m); third term via ACT accum
            vector.wait_ge(s_act, 2)
            vector.wait_ge(s_v, 2)
            vector.tensor_sub(dlog[:], logp[:], log1m[:]).then_inc(s_v, 1)   # ->4
            vector.wait_ge(s_v, 4)
            vector.tensor_mul(sbce[:], gv_ap, dlog[:]).then_inc(s_v, 1)      # ->5
            vector.wait_ge(s_v, 5)
            vector.tensor_reduce(
                parts[:, 1:2], sbce[:], axis=X, op=ADD
            ).then_inc(s_v, 1)                                               # ->6
            # PSUM [1,3] -> SBUF for the output DMA
            vector.wait_ge(s_pe, 1)
            vector.tensor_copy(res[:], acc[:]).then_inc(s_v, 1)              # ->7

        @block.tensor
        def _(tensor):
            # cross-partition reduction of all three partial columns at once
            tensor.wait_ge(s_g, 1)
            tensor.wait_ge(s_act, 3)
            tensor.wait_ge(s_v, 6)
            tensor.matmul(
                acc[:], ones[:], parts[:], start=True, stop=True
            ).then_inc(s_pe, 1)

    return nc


def _get_nc():
    if "nc" not in _CACHE:
        _CACHE["nc"] = _build_nc()
    return _CACHE["nc"]


# ---------------------------------------------------------------- host prep

def _build_in_maps(ps, pv, tg):
    P, G, M, N = _compute_matching(ps, tg)
    mp = np.take_along_axis(ps, P[..., None], axis=1)          # [B,S,10]
    mg = np.take_along_axis(tg[..., :10], G[..., None], axis=1)
    ng = np.maximum(N, 1.0)                                    # [B]
    w = np.empty((B, 1, D), np.float32)
    w[:, 0, :8] = (5.0 / (8.0 * ng * B))[:, None]
    w[:, 0, 8:] = (1.0 / (2.0 * ng * B))[:, None]
    wm = w * M[..., None]
    A = (mp * wm).astype(np.float32)
    Bt = (mg * wm).astype(np.float32)
    gv = np.ascontiguousarray(tg[..., 10:11])

    in_maps = []
    for i in range(NCORES):
        sl = slice(i * BL, (i + 1) * BL)
        x1 = np.concatenate(
            [A[sl].reshape(ROWS, LW), pv[sl].reshape(ROWS, VW)], axis=1
        )
        x2 = np.concatenate(
            [Bt[sl].reshape(ROWS, LW), gv[sl].reshape(ROWS, VW)], axis=1
        )
        in_maps.append(
            {"x1": np.ascontiguousarray(x1), "x2": np.ascontiguousarray(x2)}
        )
    return in_maps


def _combine(outs):
    """outs: per-core [1,3] = [sum weighted-L1, sum gv*dlog, sum log1m]."""
    total = np.float64(0.0)
    for r in outs:
        r = np.asarray(r, dtype=np.float64)
        total += r[0, 0] - (r[0, 1] + r[0, 2]) / float(S * B)
    return total


def kernel(pred_strokes, pred_validity, targets):
    global LAST_RESULTS
    from concourse.bass_utils import run_bass_kernel_spmd

    ps = np.asarray(pred_strokes, dtype=np.float32)
    pv = np.asarray(pred_validity, dtype=np.float32)
    tg = np.asarray(targets, dtype=np.float32)

    in_maps = _build_in_maps(ps, pv, tg)
    nc = _get_nc()
    LAST_RESULTS = run_bass_kernel_spmd(nc, in_maps, list(range(NCORES)))
    total = _combine([LAST_RESULTS.results[i]["out"] for i in range(NCORES)])
    return np.asarray(total, dtype=np.float32)
